# revision 2
# baseline (speedup 1.0000x reference)
"""TopK sparse autoencoder forward on 8 TRN2 NeuronCores — v4.

Data-parallel over tokens (1024 rows/core, no collectives). Per core:

  1. encode: pre = x @ W_enc.T + c, c = b_enc - W_enc @ b_dec folded
     host-side. x and W_enc are DMA'd directly as fp32r (same bits as
     fp32; PE rounds to FP22 internally, full rate at 512-wide moving
     dim) — selection ranks at dense-fp32r precision, no conversion
     copies.
  2. top-64 per token: DVE Max8 per 512-wide feature tile -> 576
     candidate values/row (full fp32 from PSUM) + MaxIndex on the live
     PSUM tile for in-tile positions (packed with the tile base into
     uint16 global indices). Per 8-wide round: max8 -> values,
     max_index -> candidate slots (duplicate-safe), and one
     scalar_tensor_tensor per winner resolves its global index:
        accum_out = sum((iota576 == slot_r) * gidx)
  3. decode: indirect-DMA gather (SWDGE) of the 64 active W_dec rows
     per token in bf16; the idle PE accumulates
        x_hat += diag(val_r) @ wrow_r
     into a [128, D] PSUM region (bf16 diag built on DVE from a host
     identity); final DVE add of b_dec. No dense decode GEMM, no
     activation scratch in DRAM. Level-2 + decode are interleaved per
     batch tile and their pools reuse the encode x/W area so gathers
     start while later tiles are still reducing.
"""

import os
import numpy as np
import ml_dtypes

from concourse import bass, mybir
from concourse import tile
from concourse.bass_utils import run_bass_kernel_spmd

F32 = mybir.dt.float32
F32R = mybir.dt.float32r
BF16 = mybir.dt.bfloat16
U16 = mybir.dt.uint16
U32 = mybir.dt.uint32

N_CORES = 8
B, D, F, K = 8192, 2304, 36864, 64

PT = 128          # partition tile (tokens per batch tile)
FT = 512          # encode feature tile == candidate group width
DC = 512          # decode psum chunk (free dim per matmul)


def split_waits(nc, maxw=1):
    """Walrus in this container accepts few sync-waits per instruction; Tile
    emits many. Move excess waits onto standalone same-engine no-ops."""
    for fn in nc.m.functions:
        for blk in fn.blocks:
            newinsts = []
            for inst in blk.instructions:
                si = inst.sync_info
                if si is not None and len(si.on_wait) > maxw:
                    extra = si.on_wait[:-maxw]
                    keep = si.on_wait[-maxw:]
                    for j, w in enumerate(extra):
                        nop = mybir.InstNoOp(name=f"{inst.name}-wsplit{j}", ins=[], outs=[])
                        nop.engine = inst.engine
                        nop.sync_info = mybir.SyncInfo(on_wait=[w], on_update=[])
                        newinsts.append(nop)
                    si.on_wait = keep
                newinsts.append(inst)
            blk.instructions = newinsts


def build_nc(b_loc, d, f, debug=False):
    nbt = b_loc // PT
    nd = d // PT
    nft = f // FT
    ncand = nft * 8
    ndc = (d + DC - 1) // DC
    assert ncand >= K
    assert f <= 65536  # gidx fits uint16

    nc = bass.Bass()
    xT = nc.declare_dram_parameter("xT", [d, b_loc], F32R, isOutput=False)
    wencT = nc.declare_dram_parameter("W_encT", [d, f], F32R, isOutput=False)
    wdecT = nc.declare_dram_parameter("W_decT", [f, d], BF16, isOutput=False)
    cenc = nc.declare_dram_parameter("c_enc", [f], F32R, isOutput=False)
    bdec_rep = nc.declare_dram_parameter("bdec_rep", [PT, d], F32, isOutput=False)
    ident_in = nc.declare_dram_parameter("ident", [PT, PT], BF16, isOutput=False)
    out = nc.declare_dram_parameter("out", [b_loc, d], F32, isOutput=True)

    dbg = {}
    if debug:
        dbg["t64"] = nc.declare_dram_parameter("dbg_t64", [PT, K], F32, isOutput=True)
        dbg["idxf"] = nc.declare_dram_parameter("dbg_idxf", [PT, K], F32, isOutput=True)
        dbg["vals"] = nc.declare_dram_parameter("dbg_vals", [PT, K], F32, isOutput=True)

    wencT_r = wencT.rearrange("(a p) f -> p a f", p=PT)   # [128, nd, f]
    xT_r = xT.rearrange("(a p) b -> p a b", p=PT)         # [128, nd, b_loc]
    cenc_r = cenc.rearrange("(o x) -> o x", o=1)          # [1, f]

    with tile.TileContext(nc) as tc:
        with tc.tile_pool(name="persist", bufs=1) as pp:
            ones = pp.tile([1, PT], F32R)
            nc.vector.memset(ones[:, :].bitcast(F32), 1.0)
            ident = pp.tile([PT, PT], BF16)
            nc.sync.dma_start(out=ident[:, :], in_=ident_in[:, :])
            bdec_sb = pp.tile([PT, d], F32)
            nc.sync.dma_start(out=bdec_sb[:, :], in_=bdec_rep[:, :])
            iota_c = pp.tile([PT, ncand], F32)
            nc.gpsimd.iota(iota_c[:, :], pattern=[[1, ncand]], base=0,
                           channel_multiplier=0,
                           allow_small_or_imprecise_dtypes=True)
            vals_t = [pp.tile([PT, K], F32, name=f"vals{bt}") for bt in range(nbt)]
            idxu_t = [pp.tile([PT, K], U32, name=f"idxu{bt}") for bt in range(nbt)]

            with tc.tile_pool(name="candp", bufs=1) as candp:
                cands, gidxs = [], []
                for bt in range(nbt):
                    cands.append(candp.tile([PT, ncand], F32, name=f"cand{bt}"))
                    gidxs.append(candp.tile([PT, ncand], U16, name=f"gidx{bt}"))

                # ---------------- encode + candidates ----------------
                with tc.tile_pool(name="xp", bufs=1) as xp, \
                     tc.tile_pool(name="wp", bufs=2) as wp, \
                     tc.tile_pool(name="cp", bufs=2) as cp, \
                     tc.tile_pool(name="lp", bufs=4) as lp, \
                     tc.tile_pool(name="psum_e", bufs=4, space="PSUM") as psp:

                    xh = xp.tile([PT, nd * b_loc], F32R, name="x")
                    nc.sync.dma_start(
                        out=xh[:, :].rearrange("p (a b) -> p a b", a=nd),
                        in_=xT_r[:, :, :],
                    )

                    for ft in range(nft):
                        f0 = ft * FT
                        w = wp.tile([PT, nd * FT], F32R, tag="w", name=f"w{ft}")
                        nc.sync.dma_start(
                            out=w[:, :].rearrange("p (a t) -> p a t", a=nd),
                            in_=wencT_r[:, :, f0 : f0 + FT],
                        )
                        cb = cp.tile([1, FT], F32R, tag="c", name=f"c{ft}")
                        nc.sync.dma_start(out=cb[:, :], in_=cenc_r[:, f0 : f0 + FT])

                        for bt in range(nbt):
                            ps = psp.tile([PT, FT], F32, tag="ps", name=f"ps{ft}_{bt}")
                            nc.tensor.matmul(ps[:, :], lhsT=ones[:, :], rhs=cb[:, :],
                                             start=True, stop=False)
                            for a in range(nd):
                                nc.tensor.matmul(
                                    ps[:, :],
                                    lhsT=xh[:, a * b_loc + bt * PT : a * b_loc + (bt + 1) * PT],
                                    rhs=w[:, a * FT : (a + 1) * FT],
                                    start=False, stop=(a == nd - 1),
                                )
                            c8 = cands[bt][:, ft * 8 : (ft + 1) * 8]
                            nc.vector.max(c8, ps[:, :])
                            lidx = lp.tile([PT, 8], U16, tag="lidx", name=f"li{ft}_{bt}")
                            nc.vector.max_index(lidx[:, :], c8, ps[:, :])
                            # global index = tile base + in-tile position
                            nc.vector.tensor_scalar(
                                gidxs[bt][:, ft * 8 : (ft + 1) * 8], lidx[:, :],
                                float(f0), None, mybir.AluOpType.add,
                            )

                # ---- per batch tile: top-64 + sparse decode on the PE ----
                # (pools below reuse the just-freed x/W SBUF area, so the
                #  gathers only WAR-depend on finished encode matmuls)
                with tc.tile_pool(name="t64p", bufs=2) as t64p, \
                     tc.tile_pool(name="wrp", bufs=6) as wrp, \
                     tc.tile_pool(name="diagp", bufs=4) as dgp, \
                     tc.tile_pool(name="accp", bufs=2) as accp, \
                     tc.tile_pool(name="psum_d", bufs=1, space="PSUM") as psd:
                    for bt in range(nbt):
                        t64 = t64p.tile([PT, K], F32, tag="t64", name=f"t64_{bt}")
                        idxf = t64p.tile([PT, K], F32, tag="idxf", name=f"idxf{bt}")
                        pos8 = t64p.tile([PT, 8], F32, tag="pos8", name=f"pos8{bt}")
                        posu = t64p.tile([PT, 8], U16, tag="posu", name=f"posu{bt}")
                        ext = t64p.tile([PT, ncand], F32, tag="ext", name=f"ext{bt}")
                        for r in range(K // 8):
                            t8 = t64[:, r * 8 : r * 8 + 8]
                            nc.vector.max(t8, cands[bt][:, :])
                            # slots of this round's winners (duplicate-safe)
                            nc.vector.max_index(posu[:, :], t8, cands[bt][:, :])
                            nc.vector.tensor_copy(pos8[:, :], posu[:, :])
                            for j in range(8):
                                k = r * 8 + j
                                nc.vector.scalar_tensor_tensor(
                                    ext[:, :], iota_c[:, :], pos8[:, j : j + 1],
                                    gidxs[bt][:, :],
                                    mybir.AluOpType.is_equal, mybir.AluOpType.mult,
                                    accum_out=idxf[:, k : k + 1],
                                )
                            if r < K // 8 - 1:
                                nc.vector.match_replace(
                                    cands[bt][:, :], t8, cands[bt][:, :], -1e30,
                                )
                        nc.vector.tensor_scalar(vals_t[bt][:, :], t64[:, :], 0.0,
                                                None, mybir.AluOpType.max)
                        nc.vector.tensor_copy(idxu_t[bt][:, :], idxf[:, :])
                        nc.vector.tensor_scalar(idxu_t[bt][:, :], idxu_t[bt][:, :],
                                                float(f - 1), None,
                                                mybir.AluOpType.min)
                        if debug and bt == 0:
                            nc.sync.dma_start(out=dbg["t64"][:, :], in_=t64[:, :])
                            nc.sync.dma_start(out=dbg["idxf"][:, :], in_=idxf[:, :])
                            nc.sync.dma_start(out=dbg["vals"][:, :], in_=vals_t[0][:, :])

                        # ---- decode this tile: x_hat += diag(val_r) @ wrow_r
                        pacc = psd.tile([PT, d], F32, tag="pacc", name=f"pacc{bt}")
                        for r in range(K):
                            wr = wrp.tile([PT, d], BF16, tag="wrow",
                                          name=f"wr{bt}_{r}")
                            nc.gpsimd.indirect_dma_start(
                                out=wr[:, :], out_offset=None,
                                in_=wdecT[:, :],
                                in_offset=bass.IndirectOffsetOnAxis(
                                    ap=idxu_t[bt][:, r : r + 1], axis=0),
                            )
                            diag = dgp.tile([PT, PT], BF16, tag="diag",
                                            name=f"dg{bt}_{r}")
                            nc.vector.tensor_scalar(
                                diag[:, :], ident[:, :], vals_t[bt][:, r : r + 1],
                                None, mybir.AluOpType.mult,
                            )
                            for ci in range(ndc):
                                c0, c1 = ci * DC, min((ci + 1) * DC, d)
                                nc.tensor.matmul(
                                    pacc[:, c0:c1],
                                    lhsT=diag[:, :], rhs=wr[:, c0:c1],
                                    start=(r == 0), stop=(r == K - 1),
                                )
                        acc = accp.tile([PT, d], F32, tag="acc", name=f"acc{bt}")
                        nc.vector.tensor_tensor(acc[:, :], pacc[:, :], bdec_sb[:, :],
                                                mybir.AluOpType.add)
                        nc.sync.dma_start(
                            out=out[bt * PT : (bt + 1) * PT, :],
                            in_=acc[:, :],
                        )

    return nc


def kernel(x, W_enc, b_enc, W_dec, b_dec):
    b, d = x.shape
    f = W_enc.shape[0]
    b_loc = b // N_CORES

    nc = build_nc(b_loc, d, f)
    split_waits(nc)

    bf = ml_dtypes.bfloat16
    xT = np.ascontiguousarray(np.asarray(x, np.float32).T)         # [d, b]
    wencT = np.ascontiguousarray(np.asarray(W_enc, np.float32).T)  # [d, f]
    wdecT = np.ascontiguousarray(np.asarray(W_dec, np.float32).T.astype(bf))
    c_enc = (np.asarray(b_enc, np.float32)
             - np.asarray(W_enc, np.float32) @ np.asarray(b_dec, np.float32)
             ).astype(np.float32)                                  # [f]
    bdec_rep = np.ascontiguousarray(
        np.tile(np.asarray(b_dec, np.float32)[None, :], (PT, 1)))
    ident = np.eye(PT, dtype=bf)

    in_maps = []
    for i in range(N_CORES):
        in_maps.append({
            "xT": np.ascontiguousarray(xT[:, i * b_loc : (i + 1) * b_loc]),
            "W_encT": wencT,
            "W_decT": wdecT,
            "c_enc": c_enc,
            "bdec_rep": bdec_rep,
            "ident": ident,
        })

    trace = bool(os.environ.get("BASS_TOPK_TRACE"))
    res = run_bass_kernel_spmd(nc, in_maps, list(range(N_CORES)), trace=trace)
    if trace and res.exec_time_ns is not None:
        print(f"HW exec time: {res.exec_time_ns} ns")
    shards = [res.results[i]["out"] for i in range(N_CORES)]       # [b_loc, d]
    return np.ascontiguousarray(np.concatenate(shards, axis=0))


def _numpy_ref(x, W_enc, b_enc, W_dec, b_dec, f):
    """Reference of the kernel's own algorithm (group-top8 candidates)."""
    pre = (x - b_dec) @ W_enc.T + b_enc
    g = pre.reshape(pre.shape[0], -1, FT)
    cand = -np.sort(-g, axis=2)[:, :, :8].reshape(pre.shape[0], -1)
    kth = -np.sort(-cand, axis=1)[:, K - 1]
    masked = np.maximum(pre, 0) * (pre >= kth[:, None])
    return masked @ W_dec.T + b_dec


if __name__ == "__main__":
    b_loc, d, f = 256, 256, 8192
    rng = np.random.default_rng(0)
    x = rng.standard_normal((N_CORES * b_loc, d), dtype=np.float32)
    W_enc = (rng.standard_normal((f, d), dtype=np.float32) / np.sqrt(d)).astype(np.float32)
    b_enc_ = rng.standard_normal(f, dtype=np.float32) * 0.01
    W_dec = rng.standard_normal((d, f), dtype=np.float32).astype(np.float32)
    b_dec_ = rng.standard_normal(d, dtype=np.float32) * 0.01

    got = kernel(x, W_enc, b_enc_, W_dec, b_dec_)
    want = _numpy_ref(x, W_enc, b_enc_, W_dec, b_dec_, f)
    err = np.linalg.norm(got - want) / np.linalg.norm(want)
    print("smoke rel err:", err)
